# revision 57
# baseline (speedup 1.0000x reference)
"""Self-contained TRN2 Bass kernel for nn_MultiHeadAttention_77833397338481.

kernel(**inputs) takes the FULL unsharded inputs (Q, K, V [2,1024,1024],
Wq/Wk/Wv/Wo [1024,1024], biases [1024]) and returns the FULL output
[2, 1024, 1024]. 8 NeuronCores = batch(2) x head-group(4).

Design: the Scalar-engine exp stream (16.8M exps/core @ ~1.1ns/col) is the
hard floor (~130us); everything else hides under it.
 - bf16 matmul operands everywhere, fp32 PSUM accumulation.
 - scores^T tiles stream through a 5-bank PSUM ring (A=[128,1536],
   B=[128,1024]); one fused-scale Exp per chunk -> gapless ACT stream.
 - ctx matmuls 4-way column-tiled (tile_position=(0,32j)) into one
   [128,512] PSUM bank; V padded to 32-col head slots with a ones column
   producing softmax denominators in the same matmuls.
 - packed 16-row ctx/out-proj layout halves the output projection.
 - all projections + output projection are emitted as paced "fill" work
   inside the attention phase so the PE never starves the ACT stream;
   DMAs are priority-ordered so the first exp fires ~9us in.
"""

import numpy as np
import ml_dtypes

import concourse.bacc as bacc
import concourse.mybir as mybir
import concourse.tile as tile

F32 = mybir.dt.float32
F32R = mybir.dt.float32r
BF16 = mybir.dt.bfloat16
AF = mybir.ActivationFunctionType
ALU = mybir.AluOpType

D = 1024
S = 1024
B = 2
E = 16
NQ = 4          # quads per core
NJ = 4          # heads per quad
ND = 8          # 128-row d chunks
NKB = 8         # 128-key blocks
SCALE = 1.0 / 32.0
NSL = 2 * NQ * NKB * NJ   # 256 score slices of [128 keys, 512 queries]


def bf16_np(x):
    return np.ascontiguousarray(x, np.float32).astype(ml_dtypes.bfloat16)


def round_fp32r(x):
    u = np.ascontiguousarray(x, np.float32).view(np.uint32)
    r = ((u.astype(np.uint64) + 0x800) & 0xFFFFF000).astype(np.uint32)
    return r.view(np.float32)


def chunk_of(s):
    """Global slice s -> (chunk id, position, nominal size). A chunks (even)
    hold 3 slices, B chunks (odd) hold 2."""
    pair, w = divmod(s, 5)
    if w < 3:
        return 2 * pair, w, 3
    return 2 * pair + 1, w - 3, 2


def build_nc():
    nc = bacc.Bacc("TRN2", target_bir_lowering=False, debug=False, num_devices=8)

    xq_d = nc.dram_tensor("xq", [D, S], BF16, kind="ExternalInput")
    xk_d = nc.dram_tensor("xk", [D, S], BF16, kind="ExternalInput")
    xv_d = nc.dram_tensor("xv", [D, S], BF16, kind="ExternalInput")
    wqt_d = nc.dram_tensor("wqt", [D, 512], BF16, kind="ExternalInput")
    wkt_d = nc.dram_tensor("wkt", [D, 512], BF16, kind="ExternalInput")
    wvt_d = nc.dram_tensor("wvt", [D, 272], BF16, kind="ExternalInput")
    wot_d = nc.dram_tensor("wot", [512, D], BF16, kind="ExternalInput")
    bvrow_d = nc.dram_tensor("bvrow", [1, 272], F32, kind="ExternalInput")
    bqp_d = nc.dram_tensor("bqp", [128, NQ], F32, kind="ExternalInput")
    bkp_d = nc.dram_tensor("bkp", [128, NQ], F32, kind="ExternalInput")
    ind_d = nc.dram_tensor("ind", [4, 128], F32R, kind="ExternalInput")
    out_d = nc.dram_tensor("out_part", [S, D], BF16, kind="ExternalOutput")

    with tile.TileContext(nc) as tc:
        with (
            tc.tile_pool(name="persist", bufs=1) as pp,
            tc.tile_pool(name="attn", bufs=1) as ap_,
            tc.tile_pool(name="psum", space="PSUM", bufs=1) as ps,
        ):
            # --- warm the exp table ASAP ---
            dummy = pp.tile([1, 8], F32, name="dummy")
            nc.vector.memset(dummy, 0.0)
            dummy2 = pp.tile([1, 8], F32, name="dummy2")
            nc.scalar.activation(dummy2, dummy, AF.Exp)

            # --- tiny consts ---
            ind_sb = pp.tile([4, 128], F32R, name="ind_sb")
            nc.gpsimd.dma_start(out=ind_sb, in_=ind_d[:])
            bvrow_sb = pp.tile([1, 272], F32, name="bvrow_sb")
            nc.gpsimd.dma_start(out=bvrow_sb, in_=bvrow_d[:])
            bq_sb = pp.tile([128, NQ], F32, name="bq_sb")
            nc.gpsimd.dma_start(out=bq_sb, in_=bqp_d[:])
            bk_sb = pp.tile([128, NQ], F32, name="bk_sb")
            nc.gpsimd.dma_start(out=bk_sb, in_=bkp_d[:])
            ones1 = pp.tile([1, 128], F32, name="ones1")
            nc.vector.memset(ones1, 1.0)

            # --- inputs: per-128-row-chunk tiles, one DMA each (parallel
            # queues give aggregate bandwidth; a single dma_start is only
            # ~20GB/s), ALL issued from gpsimd (~95ns vs sync ~700ns),
            # in deadline-priority order ---
            wkt = [pp.tile([128, 512], BF16, name=f"wkt{d}") for d in range(ND)]
            wqt = [pp.tile([128, 512], BF16, name=f"wqt{d}") for d in range(ND)]
            wvt = [pp.tile([128, 272], BF16, name=f"wvt{d}") for d in range(ND)]
            xk = [[pp.tile([128, 512], BF16, name=f"xk{d}_{h}") for h in range(2)]
                  for d in range(ND)]
            xq = [[pp.tile([128, 512], BF16, name=f"xq{d}_{h}") for h in range(2)]
                  for d in range(ND)]
            xv = [[pp.tile([128, 512], BF16, name=f"xv{d}_{h}") for h in range(2)]
                  for d in range(ND)]
            wot_sb = [pp.tile([128, D], BF16, name=f"wot{c}") for c in range(4)]

            def dma_x(t_sb, dram, d, h):
                nc.gpsimd.dma_start(
                    out=t_sb, in_=dram[128 * d : 128 * (d + 1), 512 * h : 512 * (h + 1)]
                )

            for d in range(ND):  # kt[0] h0 + qt[0] n0 feed the first exps
                nc.gpsimd.dma_start(out=wkt[d], in_=wkt_d[128 * d : 128 * (d + 1), :])
                dma_x(xk[d][0], xk_d, d, 0)
            for d in range(ND):
                nc.gpsimd.dma_start(out=wqt[d], in_=wqt_d[128 * d : 128 * (d + 1), :])
                dma_x(xq[d][0], xq_d, d, 0)
            for d in range(ND):  # kt h1: needed from slice 16 of t0
                dma_x(xk[d][1], xk_d, d, 1)
            for d in range(ND):  # the v path (ctx is deferred until ready)
                nc.gpsimd.dma_start(out=wvt[d], in_=wvt_d[128 * d : 128 * (d + 1), :])
                dma_x(xv[d][0], xv_d, d, 0)
            for d in range(ND):
                dma_x(xv[d][1], xv_d, d, 1)
            for d in range(ND):
                dma_x(xq[d][1], xq_d, d, 1)
            for c in range(4):
                nc.gpsimd.dma_start(out=wot_sb[c], in_=wot_d[128 * c : 128 * (c + 1), :])

            # slice helpers
            def xk_v(d, h):
                return xk[d][h]

            def xq_v(d, h):
                return xq[d][h]

            def wk_v(d, t):
                return wkt[d][:, 128 * t : 128 * (t + 1)]

            def wq_v(d, t):
                return wqt[d][:, 128 * t : 128 * (t + 1)]

            def wv_v(d):
                return wvt[d]

            def wot_v(c, dc):
                return wot_sb[c][:, 512 * dc : 512 * (dc + 1)]

            # --- persistent activations ---
            qt = [pp.tile([128, S], BF16, name=f"qt{t}") for t in range(NQ)]
            kt = [pp.tile([128, S], BF16, name=f"kt{t}") for t in range(NQ)]
            va = [pp.tile([128, 512], BF16, name=f"va{s}") for s in range(NKB)]
            for sb in range(NKB):
                nc.vector.memset(va[sb], 0.0)
            ctxp = [pp.tile([128, S], BF16, name=f"ctxp{c}") for c in range(4)]
            for c in range(4):
                nc.vector.memset(ctxp[c], 0.0)

            # --- psum ring for scores/exp (5 banks) ---
            psA = ps.tile([128, 1536], F32, name="psA", tag="psA", bufs=1)
            psB = ps.tile([128, 1024], F32, name="psB", tag="psB", bufs=1)

            # --- biasB (va bias broadcast) on the proj ring ---
            biasB_ps = ps.tile([128, 512], F32, name="biasB_ps", tag="proj", bufs=2)
            nc.tensor.matmul(biasB_ps[:, 0:272], ones1, bvrow_sb, start=True, stop=True)
            biasB = pp.tile([128, 272], F32, name="biasB")
            nc.vector.tensor_copy(biasB, biasB_ps[:, 0:272])

            # emitted-producer tracking: a consumer emitted before its
            # producer would silently read stale/garbage SBUF on hardware
            done = set()

            # ============ projection emitters ============
            def proj_qk(which, t, h):
                """q/k projection for quad t, s-half h -> qt/kt[t][:, 512h:]."""
                w, x, bias, dst = (
                    (wq_v, xq_v, bq_sb, qt) if which == "q" else (wk_v, xk_v, bk_sb, kt)
                )
                p = ps.tile([128, 512], F32, name=f"p{which}{t}{h}", tag="proj", bufs=2)
                for d in range(ND):
                    nc.tensor.matmul(
                        p,
                        w(d, t),
                        x(d, h),
                        start=(d == 0),
                        stop=(d == ND - 1),
                    )
                nc.vector.tensor_scalar(
                    dst[t][:, 512 * h : 512 * (h + 1)],
                    p,
                    bias[:, t : t + 1],
                    None,
                    ALU.add,
                )
                done.add((which, t, h))

            def proj_v(sb):
                """v projection for key block sb -> va[sb] [128 keys, 512 slots].
                Matmul runs on the compact 272-col layout (17-col head slots);
                the evac spreads slots to 32-col alignment for ctx col-tiling."""
                p = ps.tile([128, 512], F32, name=f"pv{sb}", tag="proj", bufs=2)
                h, q = divmod(sb, 4)
                for d in range(ND):
                    nc.tensor.matmul(
                        p[:, 0:272],
                        xv[d][h][:, 128 * q : 128 * (q + 1)],
                        wv_v(d),
                        start=(d == 0),
                        stop=(d == ND - 1),
                    )
                va_v = va[sb][:].rearrange("p (a b) -> p a b", b=32)[:, :, 0:17]
                p_v = p[:, 0:272].rearrange("p (a b) -> p a b", b=17)
                bb_v = biasB[:].rearrange("p (a b) -> p a b", b=17)
                nc.vector.tensor_add(va_v, p_v, bb_v)
                done.add(("va", sb))

            og_t = [ap_.tile([128, 4096], BF16, name=f"og{n}") for n in range(2)]
            og_left = [8, 8]

            def po_group(n, mt, dc):
                """output projection for token block (n, mt), d-half dc."""
                p = ps.tile([128, 512], F32, name=f"po{n}{mt}{dc}", tag="proj", bufs=2)
                for c in range(4):
                    nc.tensor.matmul(
                        p,
                        ctxp[c][:, 512 * n + 128 * mt : 512 * n + 128 * (mt + 1)],
                        wot_v(c, dc),
                        start=(c == 0),
                        stop=(c == 3),
                    )
                nc.vector.tensor_copy(
                    og_t[n][:, 1024 * mt + 512 * dc : 1024 * mt + 512 * (dc + 1)], p
                )
                og_left[n] -= 1
                if og_left[n] in (0, 4):
                    half = 0 if og_left[n] == 4 else 1
                    dst = out_d[
                        512 * n + 256 * half : 512 * n + 256 * (half + 1), :
                    ].rearrange("(m p) (c s) -> p m c s", p=128, s=512)
                    src = og_t[n][:, 2048 * half : 2048 * (half + 1)].rearrange(
                        "p (m c s) -> p m c s", m=2, c=2
                    )
                    nc.gpsimd.dma_start(out=dst, in_=src)

            # ============ lead-in: HAM warm-up burst + first projections ====
            # Dense dummy matmuls (gated only on the first tiny weight DMA)
            # interleaved with the DMA-paced first projection keep the PE
            # busy from ~0.5us so the HAM clock-gate opens to 2.4GHz before
            # the exp stream starts, instead of ~50us in.
            ones1b = pp.tile([1, 128], BF16, name="ones1b")
            nc.vector.memset(ones1b, 1.0)
            ones512b = pp.tile([1, 512], BF16, name="ones512b")
            nc.vector.memset(ones512b, 1.0)
            wu = ps.tile([128, 512], F32, name="wu", tag="proj", bufs=2)
            pk0 = ps.tile([128, 512], F32, name="pk00", tag="proj", bufs=2)
            # burst: FULL-array (K=128) matmuls gated on the first weight
            # DMA -- ~8us of sustained array activity covers a free-running
            # HAM window so the clock-gate opens before the projections.
            # (A K=1 stationary does NOT warm the HAM.)
            for _ in range(24):
                nc.tensor.matmul(
                    wu, wkt[0][:, 0:128], wkt[0][:, 0:512], start=True, stop=True
                )
            for d in range(ND):
                nc.tensor.matmul(
                    pk0,
                    wk_v(d, 0),
                    xk_v(d, 0),
                    start=(d == 0),
                    stop=(d == ND - 1),
                )
            nc.vector.tensor_scalar(
                kt[0][:, 0:512], pk0, bk_sb[:, 0:1], None, ALU.add
            )
            done.add(("k", 0, 0))
            proj_qk("q", 0, 0)

            # ============ fill schedule (deadline order) ============
            # One item pops per EVEN chunk boundary starting at c2; deferred
            # n0-ctx groups drain on the remaining boundary budget.
            fill = []
            fill.append(lambda: proj_qk("k", 0, 1))      # c2,  need c6
            fill.append(lambda: proj_qk("k", 1, 0))      # c4,  need c12
            fill.append(lambda: proj_qk("q", 1, 0))      # c6,  need c12
            fill.append(lambda: proj_v(0))               # c8
            fill.append(lambda: proj_v(1))               # c10
            fill.append(lambda: proj_v(2))               # c12
            fill.append(lambda: proj_v(3))               # c14
            fill.append(lambda: proj_qk("k", 1, 1))      # c16, need c19
            fill.append(lambda: proj_v(4))               # c18
            fill.append(lambda: proj_v(5))               # c20
            fill.append(lambda: proj_qk("k", 2, 0))      # c22, need c25
            fill.append(lambda: proj_qk("q", 2, 0))      # c24, need c25
            fill.append(lambda: proj_v(6))               # c26
            fill.append(lambda: proj_v(7))               # c28
            fill.append(lambda: proj_qk("k", 2, 1))      # c30, need c32
            fill.append(lambda: proj_qk("k", 3, 0))      # c32, need c38
            fill.append(lambda: proj_qk("q", 3, 0))      # c34, need c38
            fill.append(lambda: proj_qk("k", 3, 1))      # c36, need c44
            for t in range(NQ):
                fill.append(lambda t=t: proj_qk("q", t, 1))  # c38.., need c51+
            fill = list(reversed(fill))  # pop() from the end

            # ============ attention ============
            SLICES = [
                (n, t, i, j)
                for n in range(2)
                for t in range(NQ)
                for i in range(NKB)
                for j in range(NJ)
            ]
            from collections import deque

            LAG = 2              # chunks of delay before n1 ctx mms hit the PE fifo
            backlog = deque()    # completed chunks awaiting inline ctx emission
            chunk_tiles = {}     # chunk -> (ps tile, ex tile, width)
            chunk_members = {}   # chunk -> list of (n,t,i,j,pos)
            ctx_ps = {}          # (n,t) -> psum tile
            blocks_closed = [0]  # count of (n,t) blocks finalized
            # all n0 ctx work is deferred into a gated group queue: group
            # (t, i) = 4 col-tiled mms, releasable once proj_v(i) is emitted
            pend = deque()       # ready groups: (t, i, [(j, ext, pos), ...])
            pend_build = {}      # (t,i) -> partial member list
            pend_done = [0] * NQ # groups emitted per n0 block

            def emit_ctx(n, t, i, j, ex_t, pos):
                assert ("va", i) in done, f"ctx({n},{t},{i},{j}) before proj_v({i})"
                if (n, t) not in ctx_ps:
                    ctx_ps[(n, t)] = ps.tile(
                        [128, 512], F32, name=f"ctx{n}{t}", tag="ctx", bufs=1
                    )
                m = NJ * t + j
                nc.tensor.matmul(
                    ctx_ps[(n, t)][32 * j : 32 * (j + 1), :],
                    va[i][:, 32 * m : 32 * (m + 1)],
                    ex_t[:, 512 * pos : 512 * (pos + 1)],
                    start=(i == 0),
                    stop=(i == NKB - 1),
                    tile_position=(0, 32 * j),
                    skip_group_check=True,
                )

            def t_end(n, t):
                """softmax normalize + write packed ctxp rows."""
                cps = ctx_ps.pop((n, t))
                stage = ap_.tile([128, 512], F32, name=f"st{n}{t}", tag="stage", bufs=2)
                nc.vector.tensor_copy(stage, cps)
                den = ap_.tile([4, 512], F32, name=f"den{n}{t}", tag="den", bufs=2)
                for j in range(NJ):
                    nc.gpsimd.dma_start(
                        out=den[j : j + 1, :], in_=stage[32 * j + 16 : 32 * j + 17, :]
                    )
                with tc.high_priority(offset=-160):
                    recip = ap_.tile([4, 512], F32, name=f"rc{n}{t}", tag="recip", bufs=2)
                    scratch = ap_.tile([4, 512], F32, name=f"rs{n}{t}", tag="recip", bufs=2)
                    nc.vector.reciprocal_approx_accurate(recip, den, scratch)
                    recipr = ap_.tile([4, 512], F32R, name=f"rr{n}{t}", tag="recipr", bufs=2)
                    nc.vector.tensor_copy(recipr, recip)
                    rbw = ps.tile([128, 512], F32, name=f"rb{n}{t}", tag="proj", bufs=2)
                    nc.tensor.matmul(rbw, ind_sb, recipr, start=True, stop=True)
                    for j in range(NJ):
                        nc.vector.scalar_tensor_tensor(
                            ctxp[t][32 * j : 32 * j + 16, 512 * n : 512 * (n + 1)],
                            rbw[32 * j : 32 * j + 16, :],
                            1.0,
                            stage[32 * j : 32 * j + 16, :],
                            ALU.mult,
                            ALU.mult,
                        )

            def close_block():
                b = blocks_closed[0]
                bn, bt = divmod(b, NQ)
                t_end(bn, bt)
                blocks_closed[0] += 1
                if (bn, bt) == (0, NQ - 1):
                    # n0 ctxp done: queue n0 output projection (pops after
                    # the remaining pre-queued fill items)
                    po_items = [
                        (lambda mt=mt, dc=dc: po_group(0, mt, dc))
                        for mt in range(4)
                        for dc in range(2)
                    ]
                    fill[:0] = po_items[::-1]

            def drain_pend(maxn):
                """Emit up to maxn deferred n0 ctx groups (strict queue order;
                the head blocks until its va block's projection is emitted)."""
                while maxn > 0 and pend:
                    t2, i2, members = pend[0]
                    if ("va", i2) not in done:
                        break
                    assert blocks_closed[0] == t2, (
                        f"pend drain block {t2} but closed {blocks_closed[0]}"
                    )
                    pend.popleft()
                    for (j2, ext2, p2) in members:
                        emit_ctx(0, t2, i2, j2, ext2, p2)
                    pend_done[t2] += 1
                    if pend_done[t2] == NKB:
                        close_block()
                    maxn -= 1

            def process_ctx_batch(members_ext):
                members, ext = members_ext
                for (n2, t2, i2, j2, p2) in members:
                    if n2 == 0:
                        g = pend_build.setdefault((t2, i2), [])
                        g.append((j2, ext, p2))
                        if len(g) == NJ:
                            pend.append((t2, i2, pend_build.pop((t2, i2))))
                        continue
                    b2 = n2 * NQ + t2
                    while blocks_closed[0] < b2:
                        if blocks_closed[0] < NQ:
                            # safety: force-drain the n0 pend queue before
                            # any later block's ctx enters the PE fifo
                            before = blocks_closed[0]
                            drain_pend(10**9)
                            assert blocks_closed[0] > before, "pend drain stuck"
                        else:
                            close_block()
                    emit_ctx(n2, t2, i2, j2, ext, p2)

            def pace(c):
                return c >= 2 and c % 2 == 0

            for s, (n, t, i, j) in enumerate(SLICES):
                c, pos, size = chunk_of(s)
                width = min(size, NSL - (s - pos))
                if pos == 0:
                    pst = psA if c % 2 == 0 else psB
                    tag = "exA" if c % 2 == 0 else "exB"
                    ext = ap_.tile(
                        [128, 512 * width], BF16, name=f"ex{c}",
                        tag=f"{tag}{width}", bufs=10,
                    )
                    chunk_tiles[c] = (pst, ext, width)
                    chunk_members[c] = []
                pst, ext, width = chunk_tiles[c]
                chunk_members[c].append((n, t, i, j, pos))
                assert ("k", t, i // 4) in done, f"scores({n},{t},{i}) before kt"
                assert ("q", t, n) in done, f"scores({n},{t},{i}) before qt"
                # scores matmul into the chunk's psum slice
                nc.tensor.matmul(
                    pst[:, 512 * pos : 512 * (pos + 1)],
                    kt[t][32 * j : 32 * (j + 1), 128 * i : 128 * (i + 1)],
                    qt[t][32 * j : 32 * (j + 1), 512 * n : 512 * (n + 1)],
                    start=True,
                    stop=True,
                    tile_position=(32 * j, 0),
                )
                if pos == width - 1:
                    # chunk complete: exp it; emit lagged ctx mms; drain the
                    # deferred-n0 queue; pace fill
                    nc.scalar.activation(
                        ext, pst[:, 0 : 512 * width], AF.Exp, scale=SCALE
                    )
                    backlog.append((chunk_members.pop(c), ext))
                    if len(backlog) > LAG:
                        process_ctx_batch(backlog.popleft())
                    do_fill = bool(fill) and pace(c)
                    drain_pend(1 if do_fill else 2)
                    if do_fill:
                        fill.pop()()

            # tail: a short dependency-free burst keeps the PE dense (HAM
            # warm) while the last softmax chains drain, then flush
            wu2 = ps.tile([128, 512], F32, name="wu2", tag="proj", bufs=2)
            for _ in range(12):
                nc.tensor.matmul(wu2, ones1b, ones512b, start=True, stop=True)
            while backlog:
                process_ctx_batch(backlog.popleft())
            drain_pend(10**9)
            while blocks_closed[0] < 2 * NQ:
                close_block()
            while fill:
                fill.pop()()
            for mt in range(4):
                for dc in range(2):
                    po_group(1, mt, dc)

    nc.finalize()
    return nc


def prep_core_weights(g, Wq, bq, Wk, bk, Wv, bv, Wo):
    C0 = 256 * g
    wqt = np.zeros((D, 512), np.float32)
    wkt = np.zeros((D, 512), np.float32)
    wvt = np.zeros((D, 272), np.float32)
    bvrow = np.zeros((1, 272), np.float32)
    bqp = np.zeros((128, NQ), np.float32)
    bkp = np.zeros((128, NQ), np.float32)
    for t in range(NQ):
        for j in range(NJ):
            src = C0 + 64 * t + 16 * j
            wqt[:, 128 * t + 32 * j : 128 * t + 32 * j + E] = Wq[src : src + E, :].T
            wkt[:, 128 * t + 32 * j : 128 * t + 32 * j + E] = Wk[src : src + E, :].T
            m = NJ * t + j
            wvt[:, 17 * m : 17 * m + E] = Wv[src : src + E, :].T
            bvrow[0, 17 * m : 17 * m + E] = bv[src : src + E]
            bvrow[0, 17 * m + E] = 1.0
            bqp[32 * j : 32 * j + E, t] = bq[src : src + E]
            bkp[32 * j : 32 * j + E, t] = bk[src : src + E]
    wot = np.zeros((512, D), np.float32)
    for t in range(NQ):
        for j in range(NJ):
            src = C0 + 64 * t + 16 * j
            wot[128 * t + 32 * j : 128 * t + 32 * j + E, :] = Wo[:, src : src + E].T
    ind = np.zeros((4, 128), np.float32)
    for j in range(NJ):
        ind[j, 32 * j : 32 * j + E] = 1.0
    return {
        "wqt": bf16_np(wqt),
        "wkt": bf16_np(wkt),
        "wvt": bf16_np(wvt),
        "wot": bf16_np(wot),
        "bvrow": bvrow,
        "bqp": bqp,
        "bkp": bkp,
        "ind": round_fp32r(ind),
    }


def prep_in_maps(Q, K, V, Wq, bq, Wk, bk, Wv, bv, Wo):
    group_w = [prep_core_weights(g, Wq, bq, Wk, bk, Wv, bv, Wo) for g in range(4)]
    xt = []
    for b in range(B):
        xt.append(
            {
                "xq": bf16_np(Q[b].T),
                "xk": bf16_np(K[b].T),
                "xv": bf16_np(V[b].T),
            }
        )
    in_maps = []
    for c in range(8):
        b, g = c // 4, c % 4
        m = dict(group_w[g])
        m.update(xt[b])
        in_maps.append(m)
    return in_maps


def assemble_output(results, bo):
    out = np.zeros((B, S, D), np.float32)
    for b in range(B):
        acc = np.zeros((S, D), np.float64)
        for g in range(4):
            acc += results[4 * b + g]["out_part"].astype(np.float64)
        out[b] = (acc + bo.astype(np.float64)).astype(np.float32)
    return out


_NC_CACHE = {}


def _get_nc():
    if "nc" not in _NC_CACHE:
        _NC_CACHE["nc"] = build_nc()
    return _NC_CACHE["nc"]


def kernel(Q, K, V, Wq, bq, Wk, bk, Wv, bv, Wo, bo):
    import time

    from concourse.bass_utils import run_bass_kernel_spmd

    nc = _get_nc()
    in_maps = prep_in_maps(
        np.asarray(Q, np.float32),
        np.asarray(K, np.float32),
        np.asarray(V, np.float32),
        np.asarray(Wq, np.float32),
        np.asarray(bq, np.float32),
        np.asarray(Wk, np.float32),
        np.asarray(bk, np.float32),
        np.asarray(Wv, np.float32),
        np.asarray(bv, np.float32),
        np.asarray(Wo, np.float32),
    )
    # Retries: a first execution after NEFF load occasionally hits a
    # transient NRT_EXEC_UNIT_UNRECOVERABLE; re-running recovers.
    last = None
    for attempt in range(3):
        try:
            res = run_bass_kernel_spmd(nc, in_maps, list(range(8)))
            return assemble_output(res.results, np.asarray(bo, np.float32))
        except Exception as e:
            last = e
            time.sleep(3)
    raise last


# revision 61
# speedup vs baseline: 1.0335x; 1.0335x over previous
"""Self-contained TRN2 Bass kernel for nn_MultiHeadAttention_77833397338481.

kernel(**inputs) takes the FULL unsharded inputs (Q, K, V [2,1024,1024],
Wq/Wk/Wv/Wo [1024,1024], biases [1024]) and returns the FULL output
[2, 1024, 1024]. 8 NeuronCores = batch(2) x head-group(4).

Design: the Scalar-engine exp stream (16.8M exps/core @ ~1.1ns/col) is the
hard floor (~130us); everything else hides under it.
 - bf16 matmul operands everywhere, fp32 PSUM accumulation.
 - scores^T tiles stream through a 5-bank PSUM ring (A=[128,1536],
   B=[128,1024]); one fused-scale Exp per chunk -> gapless ACT stream.
 - ctx matmuls 4-way column-tiled (tile_position=(0,32j)) into one
   [128,512] PSUM bank; V padded to 32-col head slots with a ones column
   producing softmax denominators in the same matmuls.
 - packed 16-row ctx/out-proj layout halves the output projection.
 - all projections + output projection are emitted as paced "fill" work
   inside the attention phase so the PE never starves the ACT stream;
   DMAs are priority-ordered so the first exp fires ~9us in.
"""

import numpy as np
import ml_dtypes

import concourse.bacc as bacc
import concourse.mybir as mybir
import concourse.tile as tile

F32 = mybir.dt.float32
F32R = mybir.dt.float32r
BF16 = mybir.dt.bfloat16
AF = mybir.ActivationFunctionType
ALU = mybir.AluOpType

D = 1024
S = 1024
B = 2
E = 16
NQ = 4          # quads per core
NJ = 4          # heads per quad
ND = 8          # 128-row d chunks
NKB = 8         # 128-key blocks
SCALE = 1.0 / 32.0
NSL = 2 * NQ * NKB * NJ   # 256 score slices of [128 keys, 512 queries]


def bf16_np(x):
    return np.ascontiguousarray(x, np.float32).astype(ml_dtypes.bfloat16)


def round_fp32r(x):
    u = np.ascontiguousarray(x, np.float32).view(np.uint32)
    r = ((u.astype(np.uint64) + 0x800) & 0xFFFFF000).astype(np.uint32)
    return r.view(np.float32)


def chunk_of(s):
    """Global slice s -> (chunk id, position, nominal size). A chunks (even)
    hold 3 slices, B chunks (odd) hold 2."""
    pair, w = divmod(s, 5)
    if w < 3:
        return 2 * pair, w, 3
    return 2 * pair + 1, w - 3, 2


def build_nc():
    nc = bacc.Bacc("TRN2", target_bir_lowering=False, debug=False, num_devices=8)

    xq_d = nc.dram_tensor("xq", [D, S], BF16, kind="ExternalInput")
    xk_d = nc.dram_tensor("xk", [D, S], BF16, kind="ExternalInput")
    xv_d = nc.dram_tensor("xv", [D, S], BF16, kind="ExternalInput")
    wqt_d = nc.dram_tensor("wqt", [D, 512], BF16, kind="ExternalInput")
    wkt_d = nc.dram_tensor("wkt", [D, 512], BF16, kind="ExternalInput")
    wvt_d = nc.dram_tensor("wvt", [D, 272], BF16, kind="ExternalInput")
    wot_d = nc.dram_tensor("wot", [512, D], BF16, kind="ExternalInput")
    bvrow_d = nc.dram_tensor("bvrow", [1, 272], F32, kind="ExternalInput")
    bqp_d = nc.dram_tensor("bqp", [128, NQ], F32, kind="ExternalInput")
    bkp_d = nc.dram_tensor("bkp", [128, NQ], F32, kind="ExternalInput")
    ind_d = nc.dram_tensor("ind", [4, 128], F32R, kind="ExternalInput")
    out_d = nc.dram_tensor("out_part", [S, D], BF16, kind="ExternalOutput")

    with tile.TileContext(nc) as tc:
        with (
            tc.tile_pool(name="persist", bufs=1) as pp,
            tc.tile_pool(name="attn", bufs=1) as ap_,
            tc.tile_pool(name="psum", space="PSUM", bufs=1) as ps,
        ):
            # --- warm the exp table ASAP ---
            dummy = pp.tile([1, 8], F32, name="dummy")
            nc.vector.memset(dummy, 0.0)
            dummy2 = pp.tile([1, 8], F32, name="dummy2")
            nc.scalar.activation(dummy2, dummy, AF.Exp)

            # --- tiny consts ---
            ind_sb = pp.tile([4, 128], F32R, name="ind_sb")
            nc.gpsimd.dma_start(out=ind_sb, in_=ind_d[:])
            bvrow_sb = pp.tile([1, 272], F32, name="bvrow_sb")
            nc.gpsimd.dma_start(out=bvrow_sb, in_=bvrow_d[:])
            bq_sb = pp.tile([128, NQ], F32, name="bq_sb")
            nc.gpsimd.dma_start(out=bq_sb, in_=bqp_d[:])
            bk_sb = pp.tile([128, NQ], F32, name="bk_sb")
            nc.gpsimd.dma_start(out=bk_sb, in_=bkp_d[:])
            ones1 = pp.tile([1, 128], F32, name="ones1")
            nc.vector.memset(ones1, 1.0)

            # --- inputs: per-128-row-chunk tiles, one DMA each (parallel
            # queues give aggregate bandwidth; a single dma_start is only
            # ~20GB/s), ALL issued from gpsimd (~95ns vs sync ~700ns),
            # in deadline-priority order ---
            wkt = [pp.tile([128, 512], BF16, name=f"wkt{d}") for d in range(ND)]
            wqt = [pp.tile([128, 512], BF16, name=f"wqt{d}") for d in range(ND)]
            wvt = [pp.tile([128, 272], BF16, name=f"wvt{d}") for d in range(ND)]
            xk = [[pp.tile([128, 512], BF16, name=f"xk{d}_{h}") for h in range(2)]
                  for d in range(ND)]
            xq = [[pp.tile([128, 512], BF16, name=f"xq{d}_{h}") for h in range(2)]
                  for d in range(ND)]
            xv = [[pp.tile([128, 512], BF16, name=f"xv{d}_{h}") for h in range(2)]
                  for d in range(ND)]
            wot_sb = [pp.tile([128, D], BF16, name=f"wot{c}") for c in range(4)]

            def dma_x(t_sb, dram, d, h):
                nc.gpsimd.dma_start(
                    out=t_sb, in_=dram[128 * d : 128 * (d + 1), 512 * h : 512 * (h + 1)]
                )

            for d in range(ND):  # kt[0] h0 + qt[0] n0 feed the first exps
                nc.gpsimd.dma_start(out=wkt[d], in_=wkt_d[128 * d : 128 * (d + 1), :])
                dma_x(xk[d][0], xk_d, d, 0)
            for d in range(ND):
                nc.gpsimd.dma_start(out=wqt[d], in_=wqt_d[128 * d : 128 * (d + 1), :])
                dma_x(xq[d][0], xq_d, d, 0)
            for d in range(ND):  # kt h1: needed from slice 16 of t0
                dma_x(xk[d][1], xk_d, d, 1)
            for d in range(ND):  # the v path (ctx is deferred until ready)
                nc.gpsimd.dma_start(out=wvt[d], in_=wvt_d[128 * d : 128 * (d + 1), :])
                dma_x(xv[d][0], xv_d, d, 0)
            for d in range(ND):
                dma_x(xv[d][1], xv_d, d, 1)
            for d in range(ND):
                dma_x(xq[d][1], xq_d, d, 1)
            for c in range(4):
                nc.gpsimd.dma_start(out=wot_sb[c], in_=wot_d[128 * c : 128 * (c + 1), :])

            # slice helpers
            def xk_v(d, h):
                return xk[d][h]

            def xq_v(d, h):
                return xq[d][h]

            def wk_v(d, t):
                return wkt[d][:, 128 * t : 128 * (t + 1)]

            def wq_v(d, t):
                return wqt[d][:, 128 * t : 128 * (t + 1)]

            def wv_v(d):
                return wvt[d]

            def wot_v(c, dc):
                return wot_sb[c][:, 512 * dc : 512 * (dc + 1)]

            # --- persistent activations ---
            qt = [pp.tile([128, S], BF16, name=f"qt{t}") for t in range(NQ)]
            kt = [pp.tile([128, S], BF16, name=f"kt{t}") for t in range(NQ)]
            va = [pp.tile([128, 512], BF16, name=f"va{s}") for s in range(NKB)]
            for sb in range(NKB):
                nc.vector.memset(va[sb], 0.0)
            ctxp = [pp.tile([128, S], BF16, name=f"ctxp{c}") for c in range(4)]
            for c in range(4):
                nc.vector.memset(ctxp[c], 0.0)

            # --- psum ring for scores/exp (5 banks) ---
            psA = ps.tile([128, 1536], F32, name="psA", tag="psA", bufs=1)
            psB = ps.tile([128, 1024], F32, name="psB", tag="psB", bufs=1)

            # --- biasB (va bias broadcast) on the proj ring ---
            biasB_ps = ps.tile([128, 512], F32, name="biasB_ps", tag="proj", bufs=2)
            nc.tensor.matmul(biasB_ps[:, 0:272], ones1, bvrow_sb, start=True, stop=True)
            biasB = pp.tile([128, 272], F32, name="biasB")
            nc.vector.tensor_copy(biasB, biasB_ps[:, 0:272])

            # emitted-producer tracking: a consumer emitted before its
            # producer would silently read stale/garbage SBUF on hardware
            done = set()

            # ============ projection emitters ============
            def proj_qk(which, t, h):
                """q/k projection for quad t, s-half h -> qt/kt[t][:, 512h:]."""
                w, x, bias, dst = (
                    (wq_v, xq_v, bq_sb, qt) if which == "q" else (wk_v, xk_v, bk_sb, kt)
                )
                p = ps.tile([128, 512], F32, name=f"p{which}{t}{h}", tag="proj", bufs=2)
                for d in range(ND):
                    nc.tensor.matmul(
                        p,
                        w(d, t),
                        x(d, h),
                        start=(d == 0),
                        stop=(d == ND - 1),
                    )
                nc.vector.tensor_scalar(
                    dst[t][:, 512 * h : 512 * (h + 1)],
                    p,
                    bias[:, t : t + 1],
                    None,
                    ALU.add,
                )
                done.add((which, t, h))

            def proj_v(sb):
                """v projection for key block sb -> va[sb] [128 keys, 512 slots].
                Matmul runs on the compact 272-col layout (17-col head slots);
                the evac spreads slots to 32-col alignment for ctx col-tiling."""
                p = ps.tile([128, 512], F32, name=f"pv{sb}", tag="proj", bufs=2)
                h, q = divmod(sb, 4)
                for d in range(ND):
                    nc.tensor.matmul(
                        p[:, 0:272],
                        xv[d][h][:, 128 * q : 128 * (q + 1)],
                        wv_v(d),
                        start=(d == 0),
                        stop=(d == ND - 1),
                    )
                va_v = va[sb][:].rearrange("p (a b) -> p a b", b=32)[:, :, 0:17]
                p_v = p[:, 0:272].rearrange("p (a b) -> p a b", b=17)
                bb_v = biasB[:].rearrange("p (a b) -> p a b", b=17)
                nc.vector.tensor_add(va_v, p_v, bb_v)
                done.add(("va", sb))

            og_t = [ap_.tile([128, 4096], BF16, name=f"og{n}") for n in range(2)]
            og_left = [8, 8]

            def po_group(n, mt, dc):
                """output projection for token block (n, mt), d-half dc."""
                p = ps.tile([128, 512], F32, name=f"po{n}{mt}{dc}", tag="proj", bufs=2)
                for c in range(4):
                    nc.tensor.matmul(
                        p,
                        ctxp[c][:, 512 * n + 128 * mt : 512 * n + 128 * (mt + 1)],
                        wot_v(c, dc),
                        start=(c == 0),
                        stop=(c == 3),
                    )
                nc.vector.tensor_copy(
                    og_t[n][:, 1024 * mt + 512 * dc : 1024 * mt + 512 * (dc + 1)], p
                )
                og_left[n] -= 1
                if og_left[n] in (0, 4):
                    half = 0 if og_left[n] == 4 else 1
                    dst = out_d[
                        512 * n + 256 * half : 512 * n + 256 * (half + 1), :
                    ].rearrange("(m p) (c s) -> p m c s", p=128, s=512)
                    src = og_t[n][:, 2048 * half : 2048 * (half + 1)].rearrange(
                        "p (m c s) -> p m c s", m=2, c=2
                    )
                    nc.gpsimd.dma_start(out=dst, in_=src)

            # ============ lead-in: HAM warm-up burst + first projections ====
            # Dense dummy matmuls (gated only on the first tiny weight DMA)
            # interleaved with the DMA-paced first projection keep the PE
            # busy from ~0.5us so the HAM clock-gate opens to 2.4GHz before
            # the exp stream starts, instead of ~50us in.
            ones1b = pp.tile([1, 128], BF16, name="ones1b")
            nc.vector.memset(ones1b, 1.0)
            ones512b = pp.tile([1, 512], BF16, name="ones512b")
            nc.vector.memset(ones512b, 1.0)
            wu = ps.tile([128, 512], F32, name="wu", tag="proj", bufs=2)
            pk0 = ps.tile([128, 512], F32, name="pk00", tag="proj", bufs=2)
            # burst: FULL-array (K=128) matmuls gated on the first weight
            # DMA -- ~8us of sustained array activity covers a free-running
            # HAM window so the clock-gate opens before the projections.
            # (A K=1 stationary does NOT warm the HAM.)
            for _ in range(24):
                nc.tensor.matmul(
                    wu, wkt[0][:, 0:128], wkt[0][:, 0:512], start=True, stop=True
                )
            for d in range(ND):
                nc.tensor.matmul(
                    pk0,
                    wk_v(d, 0),
                    xk_v(d, 0),
                    start=(d == 0),
                    stop=(d == ND - 1),
                )
            nc.vector.tensor_scalar(
                kt[0][:, 0:512], pk0, bk_sb[:, 0:1], None, ALU.add
            )
            done.add(("k", 0, 0))
            proj_qk("q", 0, 0)

            # ============ fill schedule (deadline order, >=4 chunks slack) ==
            # One item pops per EVEN chunk boundary starting at c2; deferred
            # n0-ctx groups drain on the remaining boundary budget.
            fill = []
            fill.append(lambda: proj_qk("k", 0, 1))      # c2,  need c6
            fill.append(lambda: proj_qk("k", 1, 0))      # c4,  need c12
            fill.append(lambda: proj_qk("q", 1, 0))      # c6,  need c12
            fill.append(lambda: proj_qk("k", 1, 1))      # c8,  need c19
            fill.append(lambda: proj_qk("k", 2, 0))      # c10, need c25
            fill.append(lambda: proj_qk("q", 2, 0))      # c12, need c25
            fill.append(lambda: proj_v(0))               # c14
            fill.append(lambda: proj_v(1))               # c16
            fill.append(lambda: proj_v(2))               # c18
            fill.append(lambda: proj_v(3))               # c20
            fill.append(lambda: proj_qk("k", 2, 1))      # c22, need c32
            fill.append(lambda: proj_qk("k", 3, 0))      # c24, need c38
            fill.append(lambda: proj_qk("q", 3, 0))      # c26, need c38
            fill.append(lambda: proj_v(4))               # c28
            fill.append(lambda: proj_v(5))               # c30
            fill.append(lambda: proj_v(6))               # c32
            fill.append(lambda: proj_v(7))               # c34
            fill.append(lambda: proj_qk("k", 3, 1))      # c36, need c44
            for t in range(NQ):
                fill.append(lambda t=t: proj_qk("q", t, 1))  # c38.., need c51+
            fill = list(reversed(fill))  # pop() from the end

            # ============ attention ============
            SLICES = [
                (n, t, i, j)
                for n in range(2)
                for t in range(NQ)
                for i in range(NKB)
                for j in range(NJ)
            ]
            from collections import deque

            LAG = 2              # chunks of delay before n1 ctx mms hit the PE fifo
            backlog = deque()    # completed chunks awaiting inline ctx emission
            chunk_tiles = {}     # chunk -> (ps tile, ex tile, width)
            chunk_members = {}   # chunk -> list of (n,t,i,j,pos)
            ctx_ps = {}          # (n,t) -> psum tile
            blocks_closed = [0]  # count of (n,t) blocks finalized
            # all n0 ctx work is deferred into a gated group queue: group
            # (t, i) = 4 col-tiled mms, releasable once proj_v(i) is emitted
            pend = deque()       # ready groups: (t, i, [(j, ext, pos), ...])
            pend_build = {}      # (t,i) -> partial member list
            pend_done = [0] * NQ # groups emitted per n0 block

            def emit_ctx(n, t, i, j, ex_t, pos):
                assert ("va", i) in done, f"ctx({n},{t},{i},{j}) before proj_v({i})"
                if (n, t) not in ctx_ps:
                    ctx_ps[(n, t)] = ps.tile(
                        [128, 512], F32, name=f"ctx{n}{t}", tag="ctx", bufs=1
                    )
                m = NJ * t + j
                nc.tensor.matmul(
                    ctx_ps[(n, t)][32 * j : 32 * (j + 1), :],
                    va[i][:, 32 * m : 32 * (m + 1)],
                    ex_t[:, 512 * pos : 512 * (pos + 1)],
                    start=(i == 0),
                    stop=(i == NKB - 1),
                    tile_position=(0, 32 * j),
                    skip_group_check=True,
                )

            def t_end(n, t):
                """softmax normalize + write packed ctxp rows."""
                cps = ctx_ps.pop((n, t))
                stage = ap_.tile([128, 512], F32, name=f"st{n}{t}", tag="stage", bufs=2)
                den = ap_.tile([4, 512], F32, name=f"den{n}{t}", tag="den", bufs=2)
                nc.vector.tensor_copy(stage, cps)
                for j in range(NJ):
                    nc.gpsimd.dma_start(
                        out=den[j : j + 1, :], in_=stage[32 * j + 16 : 32 * j + 17, :]
                    )
                with tc.high_priority(offset=-160):
                    recip = ap_.tile([4, 512], F32, name=f"rc{n}{t}", tag="recip", bufs=2)
                    scratch = ap_.tile([4, 512], F32, name=f"rs{n}{t}", tag="recip", bufs=2)
                    nc.vector.reciprocal_approx_accurate(recip, den, scratch)
                    recipr = ap_.tile([4, 512], F32R, name=f"rr{n}{t}", tag="recipr", bufs=2)
                    nc.vector.tensor_copy(recipr, recip)
                    rbw = ps.tile([128, 512], F32, name=f"rb{n}{t}", tag="proj", bufs=2)
                    nc.tensor.matmul(rbw, ind_sb, recipr, start=True, stop=True)
                    for j in range(NJ):
                        nc.vector.scalar_tensor_tensor(
                            ctxp[t][32 * j : 32 * j + 16, 512 * n : 512 * (n + 1)],
                            rbw[32 * j : 32 * j + 16, :],
                            1.0,
                            stage[32 * j : 32 * j + 16, :],
                            ALU.mult,
                            ALU.mult,
                        )

            def close_block():
                b = blocks_closed[0]
                bn, bt = divmod(b, NQ)
                t_end(bn, bt)
                blocks_closed[0] += 1
                if (bn, bt) == (0, NQ - 1):
                    # n0 ctxp done: queue n0 output projection (pops after
                    # the remaining pre-queued fill items)
                    po_items = [
                        (lambda mt=mt, dc=dc: po_group(0, mt, dc))
                        for mt in range(4)
                        for dc in range(2)
                    ]
                    fill[:0] = po_items[::-1]

            def drain_pend(maxn):
                """Emit up to maxn deferred n0 ctx groups (strict queue order;
                the head blocks until its va block's projection is emitted)."""
                while maxn > 0 and pend:
                    t2, i2, members = pend[0]
                    if ("va", i2) not in done:
                        break
                    assert blocks_closed[0] == t2, (
                        f"pend drain block {t2} but closed {blocks_closed[0]}"
                    )
                    pend.popleft()
                    for (j2, ext2, p2) in members:
                        emit_ctx(0, t2, i2, j2, ext2, p2)
                    pend_done[t2] += 1
                    if pend_done[t2] == NKB:
                        close_block()
                    maxn -= 1

            def process_ctx_batch(members_ext):
                members, ext = members_ext
                for (n2, t2, i2, j2, p2) in members:
                    if n2 == 0:
                        g = pend_build.setdefault((t2, i2), [])
                        g.append((j2, ext, p2))
                        if len(g) == NJ:
                            pend.append((t2, i2, pend_build.pop((t2, i2))))
                        continue
                    b2 = n2 * NQ + t2
                    while blocks_closed[0] < b2:
                        if blocks_closed[0] < NQ:
                            # safety: force-drain the n0 pend queue before
                            # any later block's ctx enters the PE fifo
                            before = blocks_closed[0]
                            drain_pend(10**9)
                            assert blocks_closed[0] > before, "pend drain stuck"
                        else:
                            close_block()
                    emit_ctx(n2, t2, i2, j2, ext, p2)

            def pace(c):
                return c >= 2 and c % 2 == 0

            for s, (n, t, i, j) in enumerate(SLICES):
                c, pos, size = chunk_of(s)
                width = min(size, NSL - (s - pos))
                if pos == 0:
                    pst = psA if c % 2 == 0 else psB
                    tag = "exA" if c % 2 == 0 else "exB"
                    ext = ap_.tile(
                        [128, 512 * width], BF16, name=f"ex{c}",
                        tag=f"{tag}{width}", bufs=10,
                    )
                    chunk_tiles[c] = (pst, ext, width)
                    chunk_members[c] = []
                pst, ext, width = chunk_tiles[c]
                chunk_members[c].append((n, t, i, j, pos))
                assert ("k", t, i // 4) in done, f"scores({n},{t},{i}) before kt"
                assert ("q", t, n) in done, f"scores({n},{t},{i}) before qt"
                # scores matmul into the chunk's psum slice
                nc.tensor.matmul(
                    pst[:, 512 * pos : 512 * (pos + 1)],
                    kt[t][32 * j : 32 * (j + 1), 128 * i : 128 * (i + 1)],
                    qt[t][32 * j : 32 * (j + 1), 512 * n : 512 * (n + 1)],
                    start=True,
                    stop=True,
                    tile_position=(32 * j, 0),
                )
                if pos == width - 1:
                    # chunk complete: exp it; emit lagged ctx mms; drain the
                    # deferred-n0 queue; pace fill
                    nc.scalar.activation(
                        ext, pst[:, 0 : 512 * width], AF.Exp, scale=SCALE
                    )
                    backlog.append((chunk_members.pop(c), ext))
                    if len(backlog) > LAG:
                        process_ctx_batch(backlog.popleft())
                    do_fill = bool(fill) and pace(c)
                    drain_pend(1 if do_fill else 2)
                    if do_fill:
                        fill.pop()()

            # tail: a short full-array burst keeps the HAM clock-gate open
            # while the last softmax chains drain, then flush
            wu2 = ps.tile([128, 512], F32, name="wu2", tag="proj", bufs=2)
            for _ in range(14):
                nc.tensor.matmul(
                    wu2, wot_sb[0][:, 0:128], wot_sb[0][:, 0:512],
                    start=True, stop=True,
                )
            while backlog:
                process_ctx_batch(backlog.popleft())
            drain_pend(10**9)
            while blocks_closed[0] < 2 * NQ:
                close_block()
            while fill:
                fill.pop()()
            for mt in range(4):
                for dc in range(2):
                    po_group(1, mt, dc)

    nc.finalize()
    return nc


def prep_core_weights(g, Wq, bq, Wk, bk, Wv, bv, Wo):
    C0 = 256 * g
    wqt = np.zeros((D, 512), np.float32)
    wkt = np.zeros((D, 512), np.float32)
    wvt = np.zeros((D, 272), np.float32)
    bvrow = np.zeros((1, 272), np.float32)
    bqp = np.zeros((128, NQ), np.float32)
    bkp = np.zeros((128, NQ), np.float32)
    for t in range(NQ):
        for j in range(NJ):
            src = C0 + 64 * t + 16 * j
            wqt[:, 128 * t + 32 * j : 128 * t + 32 * j + E] = Wq[src : src + E, :].T
            wkt[:, 128 * t + 32 * j : 128 * t + 32 * j + E] = Wk[src : src + E, :].T
            m = NJ * t + j
            wvt[:, 17 * m : 17 * m + E] = Wv[src : src + E, :].T
            bvrow[0, 17 * m : 17 * m + E] = bv[src : src + E]
            bvrow[0, 17 * m + E] = 1.0
            bqp[32 * j : 32 * j + E, t] = bq[src : src + E]
            bkp[32 * j : 32 * j + E, t] = bk[src : src + E]
    wot = np.zeros((512, D), np.float32)
    for t in range(NQ):
        for j in range(NJ):
            src = C0 + 64 * t + 16 * j
            wot[128 * t + 32 * j : 128 * t + 32 * j + E, :] = Wo[:, src : src + E].T
    ind = np.zeros((4, 128), np.float32)
    for j in range(NJ):
        ind[j, 32 * j : 32 * j + E] = 1.0
    return {
        "wqt": bf16_np(wqt),
        "wkt": bf16_np(wkt),
        "wvt": bf16_np(wvt),
        "wot": bf16_np(wot),
        "bvrow": bvrow,
        "bqp": bqp,
        "bkp": bkp,
        "ind": round_fp32r(ind),
    }


def prep_in_maps(Q, K, V, Wq, bq, Wk, bk, Wv, bv, Wo):
    group_w = [prep_core_weights(g, Wq, bq, Wk, bk, Wv, bv, Wo) for g in range(4)]
    xt = []
    for b in range(B):
        xt.append(
            {
                "xq": bf16_np(Q[b].T),
                "xk": bf16_np(K[b].T),
                "xv": bf16_np(V[b].T),
            }
        )
    in_maps = []
    for c in range(8):
        b, g = c // 4, c % 4
        m = dict(group_w[g])
        m.update(xt[b])
        in_maps.append(m)
    return in_maps


def assemble_output(results, bo):
    out = np.zeros((B, S, D), np.float32)
    for b in range(B):
        acc = np.zeros((S, D), np.float64)
        for g in range(4):
            acc += results[4 * b + g]["out_part"].astype(np.float64)
        out[b] = (acc + bo.astype(np.float64)).astype(np.float32)
    return out


_NC_CACHE = {}


def _get_nc():
    if "nc" not in _NC_CACHE:
        _NC_CACHE["nc"] = build_nc()
    return _NC_CACHE["nc"]


def kernel(Q, K, V, Wq, bq, Wk, bk, Wv, bv, Wo, bo):
    import time

    from concourse.bass_utils import run_bass_kernel_spmd

    nc = _get_nc()
    in_maps = prep_in_maps(
        np.asarray(Q, np.float32),
        np.asarray(K, np.float32),
        np.asarray(V, np.float32),
        np.asarray(Wq, np.float32),
        np.asarray(bq, np.float32),
        np.asarray(Wk, np.float32),
        np.asarray(bk, np.float32),
        np.asarray(Wv, np.float32),
        np.asarray(bv, np.float32),
        np.asarray(Wo, np.float32),
    )
    # Retries: a first execution after NEFF load occasionally hits a
    # transient NRT_EXEC_UNIT_UNRECOVERABLE; re-running recovers.
    last = None
    for attempt in range(3):
        try:
            res = run_bass_kernel_spmd(nc, in_maps, list(range(8)))
            return assemble_output(res.results, np.asarray(bo, np.float32))
        except Exception as e:
            last = e
            time.sleep(3)
    raise last
